# revision 14
# baseline (speedup 1.0000x reference)
"""Distributed Trainium2 kernel for nn_Attention_30262339567666 (v2).

Multi-head causal attention with RoPE: B=2, S=2048, HID=2048, NH=16, HD=128.

Sharding: tensor-parallel over heads across 8 cores (2 heads/core), o_proj
column-parallel after an AllGather of the per-head context.

Changes over the original baseline (informed by NTFF trace analysis —
the PE streams near-perfectly at the power-throttled 1.95GHz clock, so the
wins are removing PE work and closing scheduling stalls):
  - The softmax denominator path is off the PE entirely: fp16 DVE adds
    reduce the exp tiles, gpsimd partition_all_reduce sums across
    partitions (broadcasting the result), DVE computes the reciprocal
    in-place. Replaces the ones-matmul quad sums and the slow f32r rank-1
    broadcast matmuls (~27us of PE).
  - Attention epilogues are pipelined across query blocks: the normalize
    chain is emitted one block late and its ctx DMA + AllGather trigger two
    blocks late, so neither the in-order PE queue nor the SP DMA sequencer
    (4-deep wait queue) ever sits on an unresolved producer chain.
  - AllGathers run per (batch, head, 1024-token half) — 8 smaller AGs that
    overlap the b=1 attention and o_proj streams; all of batch-0 o_proj is
    scheduled after attention(1,1) to fill the last AG windows.
  - Prologue DMAs are chunk-interleaved (wq/xblk) so the first projection
    chain starts early; xblk loads use 2 chunks to limit parked DMA waits.
  - exp/v tiles are fp16 (same speed, better precision than bf16).
"""

import sys

sys.path.insert(0, "/opt/trn_rl_repo")

import numpy as np
import ml_dtypes

import concourse.bass as bass
import concourse.tile as tile
from concourse import bacc, bass_isa, mybir
from concourse.bass_utils import run_bass_kernel_spmd

# Problem dims
B, S, HID, NH = 2, 2048, 2048, 16
HD = HID // NH           # 128
NC = 8                   # cores
HPC = NH // NC           # heads per core = 2
DL = HPC * HD            # local head dims = 256
T = B * S                # 4096 tokens
NEG = -1e9

BF16 = mybir.dt.bfloat16
F16 = mybir.dt.float16
F32 = mybir.dt.float32
AF = mybir.ActivationFunctionType

TOK_BLK = 512            # token block for projections / o_proj
N_TB = T // TOK_BLK      # 8
QB = 512                 # query block in attention
KB = 128                 # key tile (partition dim)
NQB = S // QB            # 4 query blocks per (batch, head)

LAST_EXEC_NS = None

_CACHE = {}


def _rope_tables():
    """cos/sin tables, transposed to [HD, S], matching reference numerics."""
    inv_freq = 1.0 / (10000.0 ** (np.arange(0, HD, 2, dtype=np.float64) / HD))
    t = np.arange(S, dtype=np.float64)
    freqs = np.outer(t, inv_freq)                 # [S, HD/2]
    emb = np.concatenate([freqs, freqs], axis=-1)  # [S, HD]
    cos = np.cos(emb).astype(np.float32)
    sin = np.sin(emb).astype(np.float32)
    return np.ascontiguousarray(cos.T), np.ascontiguousarray(sin.T)  # [HD, S]


def _build():
    nc = bacc.Bacc("TRN2", target_bir_lowering=False, debug=False,
                   enable_asserts=False, num_devices=NC)

    xT = nc.dram_tensor("xT", [128, N_TB, HID // 128, TOK_BLK], BF16,
                        kind="ExternalInput").ap()
    wqT = nc.dram_tensor("wqT", [128, HID // 128, DL], BF16, kind="ExternalInput").ap()
    wkT = nc.dram_tensor("wkT", [128, HID // 128, DL], BF16, kind="ExternalInput").ap()
    wvT = nc.dram_tensor("wvT", [128, HID // 128, DL], BF16, kind="ExternalInput").ap()
    woT = nc.dram_tensor("woT", [128, HID // 128, DL], BF16, kind="ExternalInput").ap()
    cosT = nc.dram_tensor("cosT", [HD, S], BF16, kind="ExternalInput").ap()
    sinT = nc.dram_tensor("sinT", [HD, S], BF16, kind="ExternalInput").ap()
    masks = nc.dram_tensor("masks", [KB, KB], F16, kind="ExternalInput").ap()
    out = nc.dram_tensor("out", [DL, T], F32, kind="ExternalOutput").ap()

    KT = HID // 128  # 16 contraction tiles

    from contextlib import ExitStack
    with tile.TileContext(nc) as tc, ExitStack() as ctx:
        sing = ctx.enter_context(tc.tile_pool(name="sing", bufs=1))
        xpool = ctx.enter_context(tc.tile_pool(name="xpool", bufs=2))
        cpool = ctx.enter_context(tc.tile_pool(name="cpool", bufs=6))
        rpool = ctx.enter_context(tc.tile_pool(name="rpool", bufs=3))
        epool = ctx.enter_context(tc.tile_pool(name="epool", bufs=8))
        qpool = ctx.enter_context(tc.tile_pool(name="qpool", bufs=4))
        spool = ctx.enter_context(tc.tile_pool(name="spool", bufs=2))
        ps_proj = ctx.enter_context(tc.tile_pool(name="ps_proj", bufs=3, space="PSUM"))
        ps_score = ctx.enter_context(tc.tile_pool(name="ps_score", bufs=3, space="PSUM"))
        ps_ctx = ctx.enter_context(tc.tile_pool(name="ps_ctx", bufs=2, space="PSUM"))
        dram = ctx.enter_context(tc.tile_pool(name="dram", bufs=1, space="DRAM"))

        # ---- resident SBUF tensors ----
        wq_sb = sing.tile([128, KT, DL], BF16)
        wk_sb = sing.tile([128, KT, DL], BF16)
        wv_sb = sing.tile([128, KT, DL], BF16)
        wo_sb = sing.tile([128, KT, DL], BF16)
        cos_sb = sing.tile([HD, S], BF16)
        sin_sb = sing.tile([HD, S], BF16)
        mask_sb = sing.tile([KB, KB], F16)
        qT_sb = sing.tile([128, HPC, T], BF16)
        kT_sb = sing.tile([128, HPC, T], BF16)
        v_sb = sing.tile([128, HPC, T // 128, HD], F16)

        ctx_loc = [[[dram.tile([HD, 2 * QB], BF16, name=f"ctx_loc{b}_{m}_{h}")
                     for h in range(2)] for m in range(HPC)] for b in range(B)]
        ctx_g = [[[dram.tile([NC * HD, 2 * QB], BF16, addr_space="Shared",
                             name=f"ctx_g{b}_{m}_{h}") for h in range(2)]
                  for m in range(HPC)] for b in range(B)]
        ctx_locq = [dram.tile([HD, QB], BF16, name=f"ctx_locq{q}")
                    for q in range(2)]
        ctx_gq = [dram.tile([NC * HD, QB], BF16, addr_space="Shared",
                            name=f"ctx_gq{q}") for q in range(2)]
        ctx_first_dma = {}
        ctx_last_dma = {}

        # ---------------- phase 1: q/k/v projections + RoPE ----------------
        def load_xblk(tb):
            xblk = xpool.tile([128, KT, TOK_BLK], BF16, name="xblk", tag="xblk")
            for ch in range(2):
                nc.sync.dma_start(out=xblk[:, 8 * ch:8 * ch + 8, :],
                                  in_=xT[:, tb, 8 * ch:8 * ch + 8, :])
            return xblk

        def phase1_block(tb, xblk=None, inject=None):
            pos0 = (tb % (S // TOK_BLK)) * TOK_BLK   # position within batch
            t0 = tb * TOK_BLK                        # global token offset
            if xblk is None:
                xblk = load_xblk(tb)

            # qT / kT with RoPE epilogue
            first = True
            for w_sb, dst in ((wq_sb, qT_sb), (wk_sb, kT_sb)):
                for m in range(HPC):
                    psq = ps_proj.tile([128, TOK_BLK], F32, name="psq", tag="proj")
                    for kt in range(KT):
                        nc.tensor.matmul(
                            psq[:],
                            w_sb[:, kt, m * 128:(m + 1) * 128],
                            xblk[:, kt, :],
                            start=(kt == 0), stop=(kt == KT - 1),
                        )
                    if first and inject is not None:
                        inject()
                    first = False
                    # RoPE: out = psq * cos + rotate_half(psq) * sin
                    rt = rpool.tile([128, TOK_BLK], BF16, name="rt", tag="rt")
                    t1 = rpool.tile([128, TOK_BLK], BF16, name="t1", tag="t1")
                    h = HD // 2
                    nc.scalar.activation(out=rt[0:h, :], in_=psq[h:HD, :],
                                         func=AF.Copy, scale=-1.0)
                    nc.scalar.activation(out=rt[h:HD, :], in_=psq[0:h, :],
                                         func=AF.Copy)
                    cs = cos_sb[:, pos0:pos0 + TOK_BLK]
                    sn = sin_sb[:, pos0:pos0 + TOK_BLK]
                    nc.vector.tensor_mul(t1, psq[:], cs)
                    nc.vector.tensor_mul(rt, rt, sn)
                    nc.vector.tensor_add(dst[:, m, t0:t0 + TOK_BLK], t1, rt)

            # v in natural layout [tokens, d], fp16
            for pair in range(2):
                psv = ps_proj.tile([128, 512], F32, name="psv", tag="proj")
                for half in range(2):
                    mt = pair * 2 + half
                    for kt in range(KT):
                        nc.tensor.matmul(
                            psv[:, half * DL:(half + 1) * DL],
                            xblk[:, kt, mt * 128:(mt + 1) * 128],
                            wv_sb[:, kt, :],
                            start=(kt == 0), stop=(kt == KT - 1),
                        )
                for half in range(2):
                    mt = pair * 2 + half
                    tt = tb * 4 + mt
                    for m in range(HPC):
                        nc.vector.tensor_copy(
                            out=v_sb[:, m, tt, :],
                            in_=psv[:, half * DL + m * HD: half * DL + (m + 1) * HD])

        # ---------------- attention for one (batch, local head) -----------
        def attention(b, m, inject=None):
            # pending: (due_qb, closure) epilogue stages. The normalize
            # chain runs one query-block late; its ctx DMA + AG trigger two
            # blocks late, so the SP sequencer never sits on a DMA whose
            # producer chain is still in flight.
            pending = []
            if inject is not None:
                pending.append((-1, inject))

            def emit_deferred(now):
                while pending and pending[0][0] <= now:
                    pending.pop(0)[1]()

            def attn_qb(qb):
                q0 = b * S + qb * QB
                nkb = 4 * (qb + 1)
                psc = ps_ctx.tile([128, QB], F32, name="psc", tag="ctx")
                exp_tiles = [None] * nkb
                quads = []

                def score_exp(kb):
                    j = kb - 4 * qb
                    lo = 128 * j if j > 0 else 0
                    pss = ps_score.tile([128, QB], F32, name="pss", tag="score")
                    nc.tensor.matmul(
                        pss[:, lo:],
                        kT_sb[:, m, b * S + kb * 128: b * S + (kb + 1) * 128],
                        qT_sb[:, m, q0 + lo:q0 + QB],
                        start=True, stop=True,
                    )
                    expT = epool.tile([128, QB], F16, name="expT", tag="expT")
                    if lo > 0:
                        nc.vector.memset(expT[:, 0:lo], 0.0)
                    if j >= 0:
                        etri = epool.tile([128, KB], F16, name="etri",
                                          tag="etri")
                        nc.scalar.activation(out=etri,
                                             in_=pss[:, lo:lo + KB],
                                             func=AF.Exp)
                        nc.vector.tensor_mul(expT[:, lo:lo + KB], etri,
                                             mask_sb[:])
                        if lo + KB < QB:
                            nc.scalar.activation(out=expT[:, lo + KB:],
                                                 in_=pss[:, lo + KB:],
                                                 func=AF.Exp)
                    else:
                        nc.scalar.activation(out=expT[:, lo:], in_=pss[:, lo:],
                                             func=AF.Exp)
                    exp_tiles[kb] = expT

                def pv(kb):
                    j = kb - 4 * qb
                    lo = 128 * j if j > 0 else 0
                    nc.tensor.matmul(
                        psc[:, lo:],
                        v_sb[:, m, b * 16 + kb, :],
                        exp_tiles[kb][:, lo:],
                        start=(kb == 0), stop=(kb == nkb - 1),
                    )

                def quad(i):
                    # fp16 in-place accumulation of 4 exp tiles on DVE
                    qd = qpool.tile([128, QB], F16, name="qd", tag="qd")
                    with nc.allow_low_precision(reason="fp16 denom tree"):
                        nc.vector.tensor_add(qd, exp_tiles[4 * i],
                                             exp_tiles[4 * i + 1])
                        nc.vector.tensor_add(qd, qd, exp_tiles[4 * i + 2])
                        nc.vector.tensor_add(qd, qd, exp_tiles[4 * i + 3])
                    quads.append(qd)

                score_exp(0)
                for kb in range(1, nkb):
                    score_exp(kb)
                    if kb == 2:
                        emit_deferred(qb)
                    pv(kb - 1)
                    if kb % 4 == 3:
                        quad(kb // 4)
                pv(nkb - 1)

                def epilogue(qb=qb, psc=psc, quads=quads):
                    with nc.allow_low_precision(reason="fp16 denom tree"):
                        acc = quads[0]
                        for qd in quads[1:]:
                            nc.vector.tensor_add(acc, acc, qd)
                    pa = spool.tile([128, QB], F32, name="pa", tag="pa")
                    nc.vector.tensor_copy(out=pa, in_=acc)
                    den = spool.tile([128, QB], F32, name="den", tag="den")
                    nc.gpsimd.partition_all_reduce(den[:], pa[:], 128,
                                                   bass_isa.ReduceOp.add)
                    with nc.allow_low_precision(reason="softmax recip"):
                        nc.vector.reciprocal_approx_fast(out=den[:],
                                                         in_=den[:])
                    ctxt = rpool.tile([128, QB], BF16, name="ctxs", tag="ctx_sb")
                    nc.vector.tensor_mul(ctxt, psc[:], den[:])

                    def dma_stage(qb=qb, ctxt=ctxt):
                        if b == 1 and m == 1 and qb >= 2:
                            dma = nc.sync.dma_start(
                                out=ctx_locq[qb - 2][:], in_=ctxt)
                            ctx_last_dma[(b, m)] = dma
                            nc.gpsimd.collective_compute(
                                "AllGather", mybir.AluOpType.bypass,
                                replica_groups=[list(range(NC))],
                                ins=[ctx_locq[qb - 2].opt()],
                                outs=[ctx_gq[qb - 2].opt()])
                            return
                        dma = nc.sync.dma_start(
                            out=ctx_loc[b][m][qb // 2][:, (qb % 2) * QB:
                                                       (qb % 2 + 1) * QB],
                            in_=ctxt)
                        ctx_first_dma.setdefault((b, m), dma)
                        ctx_last_dma[(b, m)] = dma
                        if qb % 2 == 1:
                            nc.gpsimd.collective_compute(
                                "AllGather", mybir.AluOpType.bypass,
                                replica_groups=[list(range(NC))],
                                ins=[ctx_loc[b][m][qb // 2].opt()],
                                outs=[ctx_g[b][m][qb // 2].opt()])

                    pending.append((qb + 2, dma_stage))

                pending.append((qb + 1, epilogue))

            for qb in range(NQB):
                attn_qb(qb)

            def fin():
                emit_deferred(10 ** 9)

            return fin

        # ---------------- phase 2: o_proj ----------------------------------
        c_half = {}

        def phase2_prefetch(tb, mh, eng=None):
            b = tb // (S // TOK_BLK)
            pos0 = (tb % (S // TOK_BLK)) * TOK_BLK
            ch = cpool.tile([128, KT // 2, TOK_BLK], BF16, name="ch", tag="ch")
            eng = eng or nc.sync
            if tb >= 6 and mh == 1:
                g_r = ctx_gq[tb - 6].rearrange("(t p) n -> p t n", p=128)
                for c0 in (0, KT // 4):
                    eng.dma_start(out=ch[:, c0:c0 + KT // 4, :],
                                  in_=g_r[:, c0:c0 + KT // 4, :])
            else:
                half, off = pos0 // (2 * QB), pos0 % (2 * QB)
                g_r = ctx_g[b][mh][half].rearrange("(t p) n -> p t n", p=128)
                for c0 in (0, KT // 4):
                    eng.dma_start(
                        out=ch[:, c0:c0 + KT // 4, :],
                        in_=g_r[:, c0:c0 + KT // 4, off:off + TOK_BLK])
            c_half[(tb, mh)] = ch

        def phase2_compute(tb, inject=None):
            t0 = tb * TOK_BLK
            for m in range(HPC):
                pso = ps_proj.tile([128, TOK_BLK], F32, name="pso", tag="proj")
                i = 0
                for mh in range(2):
                    ch = c_half[(tb, mh)]
                    for j in range(KT // 2):
                        kt = 2 * j + mh
                        nc.tensor.matmul(
                            pso[:],
                            wo_sb[:, kt, m * 128:(m + 1) * 128],
                            ch[:, j, :],
                            start=(i == 0), stop=(i == KT - 1),
                        )
                        i += 1
                    if m == 0 and mh == 0 and inject is not None:
                        inject()
                osb = spool.tile([128, TOK_BLK], F32, name="osb", tag="osb")
                nc.scalar.activation(out=osb, in_=pso[:], func=AF.Copy)
                nc.sync.dma_start(out=out[m * 128:(m + 1) * 128, t0:t0 + TOK_BLK],
                                  in_=osb)

        # ---------------- emission order -----------------------------------
        # prologue: interleave wq/xblk0 chunks so the first projection chain
        # starts after ~0.75MB of DMA instead of ~5MB.
        xblk0 = xpool.tile([128, KT, TOK_BLK], BF16, name="xblk", tag="xblk")
        # critical first chunks on the ACT HWDGE queues: the SP sequencer
        # spends ~7us on startup protocol before its first DMA issues; ACT
        # is idle at kernel start so these fire earlier.
        nc.scalar.dma_start(out=wq_sb[:, 0:2, :], in_=wqT[:, 0:2, :])
        nc.scalar.dma_start(out=xblk0[:, 0:2, :], in_=xT[:, 0, 0:2, :])
        nc.scalar.dma_start(out=wq_sb[:, 2:8, :], in_=wqT[:, 2:8, :])
        nc.scalar.dma_start(out=xblk0[:, 2:8, :], in_=xT[:, 0, 2:8, :])
        nc.sync.dma_start(out=wq_sb[:, 8:16, :], in_=wqT[:, 8:16, :])
        nc.sync.dma_start(out=xblk0[:, 8:16, :], in_=xT[:, 0, 8:16, :])
        for chk in range(4):
            nc.sync.dma_start(out=wk_sb[:, 4 * chk:4 * chk + 4, :],
                              in_=wkT[:, 4 * chk:4 * chk + 4, :])
        nc.sync.dma_start(out=cos_sb, in_=cosT)
        nc.sync.dma_start(out=sin_sb, in_=sinT)
        xblk1 = load_xblk(1)
        nc.sync.dma_start(out=wv_sb, in_=wvT)
        phase1_block(0, xblk0)
        phase1_block(1, xblk1)
        phase1_block(2)
        phase1_block(3)
        nc.sync.dma_start(out=mask_sb, in_=masks)
        nc.sync.dma_start(out=wo_sb, in_=woT)
        fin00 = attention(0, 0)
        fin01 = attention(0, 1, inject=fin00)
        fin01()
        for tb in range(4, 8):
            phase1_block(tb)
        phase2_prefetch(0, 0)
        phase2_prefetch(0, 1)
        phase2_prefetch(1, 0)
        phase2_prefetch(1, 1)
        phase2_prefetch(2, 0)
        phase2_prefetch(2, 1)
        fin10 = attention(1, 0)
        fin10()
        fin11 = attention(1, 1)
        fin11()
        phase2_compute(0)
        phase2_prefetch(3, 0)
        phase2_prefetch(3, 1)
        phase2_compute(1)
        phase2_prefetch(4, 0)
        phase2_prefetch(4, 1)
        phase2_compute(2)
        phase2_prefetch(5, 0)
        phase2_prefetch(5, 1)
        phase2_compute(3)
        phase2_prefetch(6, 0)
        phase2_prefetch(6, 1)
        phase2_compute(4)
        phase2_prefetch(7, 0)
        phase2_prefetch(7, 1)
        phase2_compute(5)
        phase2_compute(6)
        phase2_compute(7)

    nc.compile()
    return nc


def kernel(hidden_states, attention_mask, wq, wk, wv, wo):
    global LAST_EXEC_NS
    bf16 = ml_dtypes.bfloat16

    hidden_states = np.asarray(hidden_states, dtype=np.float32)
    wq = np.asarray(wq, dtype=np.float32)
    wk = np.asarray(wk, dtype=np.float32)
    wv = np.asarray(wv, dtype=np.float32)
    wo = np.asarray(wo, dtype=np.float32)

    x = hidden_states.reshape(T, HID)
    # pretiled so every DMA reads contiguous per-partition chunks:
    # xT[p, tb, kt, c] = x[tb*512 + c, kt*128 + p]
    xTt = np.ascontiguousarray(
        x.reshape(N_TB, TOK_BLK, HID // 128, 128).transpose(3, 0, 2, 1)
    ).astype(bf16)
    cosT, sinT = _rope_tables()
    cosT16, sinT16 = cosT.astype(bf16), sinT.astype(bf16)
    k_idx = np.arange(KB)[:, None]
    q_idx = np.arange(KB)[None, :]
    binmask16 = (k_idx <= q_idx).astype(np.float16)

    def tile_w(w):   # [DL, HID] -> wT tiled [128, KT, DL]
        return np.ascontiguousarray(
            w.T.reshape(HID // 128, 128, DL).transpose(1, 0, 2)).astype(bf16)

    scale = np.float32(1.0 / np.sqrt(HD))
    in_maps = []
    for c in range(NC):
        rows = slice(c * DL, (c + 1) * DL)
        in_maps.append({
            "xT": xTt,
            "wqT": tile_w(wq[rows, :] * scale),
            "wkT": tile_w(wk[rows, :]),
            "wvT": tile_w(wv[rows, :]),
            "woT": tile_w(wo[rows, :]),
            "cosT": cosT16,
            "sinT": sinT16,
            "masks": binmask16,
        })

    if "nc" not in _CACHE:
        _CACHE["nc"] = _build()
    nc = _CACHE["nc"]

    res = run_bass_kernel_spmd(nc, in_maps, core_ids=list(range(NC)))
    LAST_EXEC_NS = res.exec_time_ns

    outT = np.concatenate([np.asarray(res.results[c]["out"]) for c in range(NC)],
                          axis=0)                          # [HID, T]
    return np.ascontiguousarray(outT.T).reshape(B, S, HID).astype(np.float32)


# revision 15
# speedup vs baseline: 1.0593x; 1.0593x over previous
"""Distributed Trainium2 kernel for nn_Attention_30262339567666 (v2).

Multi-head causal attention with RoPE: B=2, S=2048, HID=2048, NH=16, HD=128.

Sharding: tensor-parallel over heads across 8 cores (2 heads/core), o_proj
column-parallel after an AllGather of the per-head context.

Changes over the original baseline (informed by NTFF trace analysis —
the PE streams near-perfectly at the power-throttled 1.95GHz clock, so the
wins are removing PE work and closing scheduling stalls):
  - The softmax denominator path is off the PE entirely: fp16 DVE adds
    reduce the exp tiles, gpsimd partition_all_reduce sums across
    partitions (broadcasting the result), DVE computes the reciprocal
    in-place. Replaces the ones-matmul quad sums and the slow f32r rank-1
    broadcast matmuls (~27us of PE).
  - Attention epilogues are pipelined across query blocks: the normalize
    chain is emitted one block late and its ctx DMA + AllGather trigger two
    blocks late, so neither the in-order PE queue nor the SP DMA sequencer
    (4-deep wait queue) ever sits on an unresolved producer chain.
  - AllGathers run per (batch, head, 1024-token half) — 8 smaller AGs that
    overlap the b=1 attention and o_proj streams; all of batch-0 o_proj is
    scheduled after attention(1,1) to fill the last AG windows.
  - Prologue DMAs are chunk-interleaved (wq/xblk) so the first projection
    chain starts early; xblk loads use 2 chunks to limit parked DMA waits.
  - exp/v tiles are fp16 (same speed, better precision than bf16).
"""

import sys

sys.path.insert(0, "/opt/trn_rl_repo")

import numpy as np
import ml_dtypes

import concourse.bass as bass
import concourse.tile as tile
from concourse import bacc, bass_isa, mybir
from concourse.bass_utils import run_bass_kernel_spmd

# Problem dims
B, S, HID, NH = 2, 2048, 2048, 16
HD = HID // NH           # 128
NC = 8                   # cores
HPC = NH // NC           # heads per core = 2
DL = HPC * HD            # local head dims = 256
T = B * S                # 4096 tokens
NEG = -1e9

BF16 = mybir.dt.bfloat16
F16 = mybir.dt.float16
F32 = mybir.dt.float32
AF = mybir.ActivationFunctionType

TOK_BLK = 512            # token block for projections / o_proj
N_TB = T // TOK_BLK      # 8
QB = 512                 # query block in attention
KB = 128                 # key tile (partition dim)
NQB = S // QB            # 4 query blocks per (batch, head)

LAST_EXEC_NS = None

_CACHE = {}


def _rope_tables():
    """cos/sin tables, transposed to [HD, S], matching reference numerics."""
    inv_freq = 1.0 / (10000.0 ** (np.arange(0, HD, 2, dtype=np.float64) / HD))
    t = np.arange(S, dtype=np.float64)
    freqs = np.outer(t, inv_freq)                 # [S, HD/2]
    emb = np.concatenate([freqs, freqs], axis=-1)  # [S, HD]
    cos = np.cos(emb).astype(np.float32)
    sin = np.sin(emb).astype(np.float32)
    return np.ascontiguousarray(cos.T), np.ascontiguousarray(sin.T)  # [HD, S]


def _build():
    nc = bacc.Bacc("TRN2", target_bir_lowering=False, debug=False,
                   enable_asserts=False, num_devices=NC)

    xT = nc.dram_tensor("xT", [128, N_TB, HID // 128, TOK_BLK], BF16,
                        kind="ExternalInput").ap()
    wqT = nc.dram_tensor("wqT", [128, HID // 128, DL], BF16, kind="ExternalInput").ap()
    wkT = nc.dram_tensor("wkT", [128, HID // 128, DL], BF16, kind="ExternalInput").ap()
    wvT = nc.dram_tensor("wvT", [128, HID // 128, DL], BF16, kind="ExternalInput").ap()
    woT = nc.dram_tensor("woT", [128, HID // 128, DL], BF16, kind="ExternalInput").ap()
    cosT = nc.dram_tensor("cosT", [HD, S], BF16, kind="ExternalInput").ap()
    sinT = nc.dram_tensor("sinT", [HD, S], BF16, kind="ExternalInput").ap()
    masks = nc.dram_tensor("masks", [KB, KB], F16, kind="ExternalInput").ap()
    out = nc.dram_tensor("out", [DL, T], F32, kind="ExternalOutput").ap()

    KT = HID // 128  # 16 contraction tiles

    from contextlib import ExitStack
    with tile.TileContext(nc) as tc, ExitStack() as ctx:
        sing = ctx.enter_context(tc.tile_pool(name="sing", bufs=1))
        xpool = ctx.enter_context(tc.tile_pool(name="xpool", bufs=2))
        cpool = ctx.enter_context(tc.tile_pool(name="cpool", bufs=6))
        rpool = ctx.enter_context(tc.tile_pool(name="rpool", bufs=3))
        epool = ctx.enter_context(tc.tile_pool(name="epool", bufs=8))
        qpool = ctx.enter_context(tc.tile_pool(name="qpool", bufs=4))
        spool = ctx.enter_context(tc.tile_pool(name="spool", bufs=2))
        ps_proj = ctx.enter_context(tc.tile_pool(name="ps_proj", bufs=3, space="PSUM"))
        ps_score = ctx.enter_context(tc.tile_pool(name="ps_score", bufs=3, space="PSUM"))
        ps_ctx = ctx.enter_context(tc.tile_pool(name="ps_ctx", bufs=2, space="PSUM"))
        dram = ctx.enter_context(tc.tile_pool(name="dram", bufs=1, space="DRAM"))

        # ---- resident SBUF tensors ----
        wq_sb = sing.tile([128, KT, DL], BF16)
        wk_sb = sing.tile([128, KT, DL], BF16)
        wv_sb = sing.tile([128, KT, DL], BF16)
        wo_sb = sing.tile([128, KT, DL], BF16)
        cos_sb = sing.tile([HD, S], BF16)
        sin_sb = sing.tile([HD, S], BF16)
        mask_sb = sing.tile([KB, KB], F16)
        qT_sb = sing.tile([128, HPC, T], BF16)
        kT_sb = sing.tile([128, HPC, T], BF16)
        v_sb = sing.tile([128, HPC, T // 128, HD], F16)

        ctx_loc = [[[dram.tile([HD, 2 * QB], BF16, name=f"ctx_loc{b}_{m}_{h}")
                     for h in range(2)] for m in range(HPC)] for b in range(B)]
        ctx_g = [[[dram.tile([NC * HD, 2 * QB], BF16, addr_space="Shared",
                             name=f"ctx_g{b}_{m}_{h}") for h in range(2)]
                  for m in range(HPC)] for b in range(B)]
        ctx_locq = [dram.tile([HD, QB], BF16, name=f"ctx_locq{q}")
                    for q in range(2)]
        ctx_gq = [dram.tile([NC * HD, QB], BF16, addr_space="Shared",
                            name=f"ctx_gq{q}") for q in range(2)]
        ctx_first_dma = {}
        ctx_last_dma = {}

        # ---------------- phase 1: q/k/v projections + RoPE ----------------
        def load_xblk(tb):
            xblk = xpool.tile([128, KT, TOK_BLK], BF16, name="xblk", tag="xblk")
            for ch in range(2):
                nc.sync.dma_start(out=xblk[:, 8 * ch:8 * ch + 8, :],
                                  in_=xT[:, tb, 8 * ch:8 * ch + 8, :])
            return xblk

        def phase1_block(tb, xblk=None, inject=None):
            pos0 = (tb % (S // TOK_BLK)) * TOK_BLK   # position within batch
            t0 = tb * TOK_BLK                        # global token offset
            if xblk is None:
                xblk = load_xblk(tb)

            # qT / kT with RoPE epilogue
            first = True
            for w_sb, dst in ((wq_sb, qT_sb), (wk_sb, kT_sb)):
                for m in range(HPC):
                    psq = ps_proj.tile([128, TOK_BLK], F32, name="psq", tag="proj")
                    for kt in range(KT):
                        nc.tensor.matmul(
                            psq[:],
                            w_sb[:, kt, m * 128:(m + 1) * 128],
                            xblk[:, kt, :],
                            start=(kt == 0), stop=(kt == KT - 1),
                        )
                    if first and inject is not None:
                        inject()
                    first = False
                    # RoPE: out = psq * cos + rotate_half(psq) * sin
                    rt = rpool.tile([128, TOK_BLK], BF16, name="rt", tag="rt")
                    t1 = rpool.tile([128, TOK_BLK], BF16, name="t1", tag="t1")
                    h = HD // 2
                    nc.scalar.activation(out=rt[0:h, :], in_=psq[h:HD, :],
                                         func=AF.Copy, scale=-1.0)
                    nc.scalar.activation(out=rt[h:HD, :], in_=psq[0:h, :],
                                         func=AF.Copy)
                    cs = cos_sb[:, pos0:pos0 + TOK_BLK]
                    sn = sin_sb[:, pos0:pos0 + TOK_BLK]
                    nc.vector.tensor_mul(t1, psq[:], cs)
                    nc.vector.tensor_mul(rt, rt, sn)
                    nc.vector.tensor_add(dst[:, m, t0:t0 + TOK_BLK], t1, rt)

            # v in natural layout [tokens, d], fp16
            for pair in range(2):
                psv = ps_proj.tile([128, 512], F32, name="psv", tag="proj")
                for half in range(2):
                    mt = pair * 2 + half
                    for kt in range(KT):
                        nc.tensor.matmul(
                            psv[:, half * DL:(half + 1) * DL],
                            xblk[:, kt, mt * 128:(mt + 1) * 128],
                            wv_sb[:, kt, :],
                            start=(kt == 0), stop=(kt == KT - 1),
                        )
                for half in range(2):
                    mt = pair * 2 + half
                    tt = tb * 4 + mt
                    for m in range(HPC):
                        nc.vector.tensor_copy(
                            out=v_sb[:, m, tt, :],
                            in_=psv[:, half * DL + m * HD: half * DL + (m + 1) * HD])

        # ---------------- attention for one (batch, local head) -----------
        def attention(b, m, inject=None):
            # pending: (due_qb, closure) epilogue stages. The normalize
            # chain runs one query-block late; its ctx DMA + AG trigger two
            # blocks late, so the SP sequencer never sits on a DMA whose
            # producer chain is still in flight.
            pending = []
            if inject is not None:
                pending.append((-1, inject))

            def emit_deferred(now):
                while pending and pending[0][0] <= now:
                    pending.pop(0)[1]()

            def attn_qb(qb):
                q0 = b * S + qb * QB
                nkb = 4 * (qb + 1)
                psc = ps_ctx.tile([128, QB], F32, name="psc", tag="ctx")
                exp_tiles = [None] * nkb
                quads = []

                def score_exp(kb):
                    j = kb - 4 * qb
                    lo = 128 * j if j > 0 else 0
                    pss = ps_score.tile([128, QB], F32, name="pss", tag="score")
                    nc.tensor.matmul(
                        pss[:, lo:],
                        kT_sb[:, m, b * S + kb * 128: b * S + (kb + 1) * 128],
                        qT_sb[:, m, q0 + lo:q0 + QB],
                        start=True, stop=True,
                    )
                    expT = epool.tile([128, QB], F16, name="expT", tag="expT")
                    if lo > 0:
                        nc.vector.memset(expT[:, 0:lo], 0.0)
                    if j >= 0:
                        etri = epool.tile([128, KB], F16, name="etri",
                                          tag="etri")
                        nc.scalar.activation(out=etri,
                                             in_=pss[:, lo:lo + KB],
                                             func=AF.Exp)
                        nc.vector.tensor_mul(expT[:, lo:lo + KB], etri,
                                             mask_sb[:])
                        if lo + KB < QB:
                            nc.scalar.activation(out=expT[:, lo + KB:],
                                                 in_=pss[:, lo + KB:],
                                                 func=AF.Exp)
                    else:
                        nc.scalar.activation(out=expT[:, lo:], in_=pss[:, lo:],
                                             func=AF.Exp)
                    exp_tiles[kb] = expT

                def pv(kb):
                    j = kb - 4 * qb
                    lo = 128 * j if j > 0 else 0
                    nc.tensor.matmul(
                        psc[:, lo:],
                        v_sb[:, m, b * 16 + kb, :],
                        exp_tiles[kb][:, lo:],
                        start=(kb == 0), stop=(kb == nkb - 1),
                    )

                def quad(i):
                    # fp16 in-place accumulation of 4 exp tiles on DVE
                    qd = qpool.tile([128, QB], F16, name="qd", tag="qd")
                    with nc.allow_low_precision(reason="fp16 denom tree"):
                        nc.vector.tensor_add(qd, exp_tiles[4 * i],
                                             exp_tiles[4 * i + 1])
                        nc.vector.tensor_add(qd, qd, exp_tiles[4 * i + 2])
                        nc.vector.tensor_add(qd, qd, exp_tiles[4 * i + 3])
                    quads.append(qd)

                score_exp(0)
                for kb in range(1, nkb):
                    score_exp(kb)
                    if kb == 2:
                        emit_deferred(qb)
                    pv(kb - 1)
                    if kb % 4 == 3:
                        quad(kb // 4)
                pv(nkb - 1)

                def epilogue(qb=qb, psc=psc, quads=quads):
                    with nc.allow_low_precision(reason="fp16 denom tree"):
                        acc = quads[0]
                        for qd in quads[1:]:
                            nc.vector.tensor_add(acc, acc, qd)
                    pa = spool.tile([128, QB], F32, name="pa", tag="pa")
                    nc.vector.tensor_copy(out=pa, in_=acc)
                    den = spool.tile([128, QB], F32, name="den", tag="den")
                    nc.gpsimd.partition_all_reduce(den[:], pa[:], 128,
                                                   bass_isa.ReduceOp.add)
                    with nc.allow_low_precision(reason="softmax recip"):
                        nc.vector.reciprocal_approx_fast(out=den[:],
                                                         in_=den[:])
                    ctxt = rpool.tile([128, QB], BF16, name="ctxs", tag="ctx_sb")
                    nc.vector.tensor_mul(ctxt, psc[:], den[:])

                    def dma_stage(qb=qb, ctxt=ctxt):
                        if b == 1 and m == 1 and qb >= 2:
                            dma = nc.sync.dma_start(
                                out=ctx_locq[qb - 2][:], in_=ctxt)
                            ctx_last_dma[(b, m)] = dma
                            nc.gpsimd.collective_compute(
                                "AllGather", mybir.AluOpType.bypass,
                                replica_groups=[list(range(NC))],
                                ins=[ctx_locq[qb - 2].opt()],
                                outs=[ctx_gq[qb - 2].opt()])
                            return
                        dma = nc.sync.dma_start(
                            out=ctx_loc[b][m][qb // 2][:, (qb % 2) * QB:
                                                       (qb % 2 + 1) * QB],
                            in_=ctxt)
                        ctx_first_dma.setdefault((b, m), dma)
                        ctx_last_dma[(b, m)] = dma
                        if qb % 2 == 1:
                            nc.gpsimd.collective_compute(
                                "AllGather", mybir.AluOpType.bypass,
                                replica_groups=[list(range(NC))],
                                ins=[ctx_loc[b][m][qb // 2].opt()],
                                outs=[ctx_g[b][m][qb // 2].opt()])

                    pending.append((qb + 2, dma_stage))

                pending.append((qb + 1, epilogue))

            for qb in range(NQB):
                attn_qb(qb)

            def fin():
                emit_deferred(10 ** 9)

            return fin

        # ---------------- phase 2: o_proj ----------------------------------
        c_half = {}

        def phase2_prefetch(tb, mh, eng=None):
            b = tb // (S // TOK_BLK)
            pos0 = (tb % (S // TOK_BLK)) * TOK_BLK
            ch = cpool.tile([128, KT // 2, TOK_BLK], BF16, name="ch", tag="ch")
            eng = eng or nc.sync
            if tb >= 6 and mh == 1:
                g_r = ctx_gq[tb - 6].rearrange("(t p) n -> p t n", p=128)
                for c0 in (0, KT // 4):
                    eng.dma_start(out=ch[:, c0:c0 + KT // 4, :],
                                  in_=g_r[:, c0:c0 + KT // 4, :])
            else:
                half, off = pos0 // (2 * QB), pos0 % (2 * QB)
                g_r = ctx_g[b][mh][half].rearrange("(t p) n -> p t n", p=128)
                for c0 in (0, KT // 4):
                    eng.dma_start(
                        out=ch[:, c0:c0 + KT // 4, :],
                        in_=g_r[:, c0:c0 + KT // 4, off:off + TOK_BLK])
            c_half[(tb, mh)] = ch

        def phase2_compute(tb, inject=None):
            t0 = tb * TOK_BLK
            for m in range(HPC):
                pso = ps_proj.tile([128, TOK_BLK], F32, name="pso", tag="proj")
                i = 0
                for mh in range(2):
                    ch = c_half[(tb, mh)]
                    for j in range(KT // 2):
                        kt = 2 * j + mh
                        nc.tensor.matmul(
                            pso[:],
                            wo_sb[:, kt, m * 128:(m + 1) * 128],
                            ch[:, j, :],
                            start=(i == 0), stop=(i == KT - 1),
                        )
                        i += 1
                    if m == 0 and mh == 0 and inject is not None:
                        inject()
                osb = spool.tile([128, TOK_BLK], F32, name="osb", tag="osb")
                nc.scalar.activation(out=osb, in_=pso[:], func=AF.Copy)
                nc.sync.dma_start(out=out[m * 128:(m + 1) * 128, t0:t0 + TOK_BLK],
                                  in_=osb)

        # ---------------- emission order -----------------------------------
        # prologue: interleave wq/xblk0 chunks so the first projection chain
        # starts after ~0.75MB of DMA instead of ~5MB.
        xblk0 = xpool.tile([128, KT, TOK_BLK], BF16, name="xblk", tag="xblk")
        nc.sync.dma_start(out=wq_sb[:, 0:2, :], in_=wqT[:, 0:2, :])
        nc.sync.dma_start(out=xblk0[:, 0:2, :], in_=xT[:, 0, 0:2, :])
        nc.sync.dma_start(out=wq_sb[:, 2:8, :], in_=wqT[:, 2:8, :])
        nc.sync.dma_start(out=xblk0[:, 2:8, :], in_=xT[:, 0, 2:8, :])
        nc.sync.dma_start(out=wq_sb[:, 8:16, :], in_=wqT[:, 8:16, :])
        nc.sync.dma_start(out=xblk0[:, 8:16, :], in_=xT[:, 0, 8:16, :])
        for chk in range(4):
            nc.sync.dma_start(out=wk_sb[:, 4 * chk:4 * chk + 4, :],
                              in_=wkT[:, 4 * chk:4 * chk + 4, :])
        nc.sync.dma_start(out=cos_sb, in_=cosT)
        nc.sync.dma_start(out=sin_sb, in_=sinT)
        xblk1 = load_xblk(1)
        nc.sync.dma_start(out=wv_sb, in_=wvT)
        phase1_block(0, xblk0)
        phase1_block(1, xblk1)
        phase1_block(2)
        phase1_block(3)
        nc.sync.dma_start(out=mask_sb, in_=masks)
        nc.sync.dma_start(out=wo_sb, in_=woT)
        fin00 = attention(0, 0)
        fin01 = attention(0, 1, inject=fin00)
        fin01()
        for tb in range(4, 8):
            phase1_block(tb)
        phase2_prefetch(0, 0)
        phase2_prefetch(0, 1)
        phase2_prefetch(1, 0)
        phase2_prefetch(1, 1)
        phase2_prefetch(2, 0)
        phase2_prefetch(2, 1)
        fin10 = attention(1, 0)
        fin10()
        fin11 = attention(1, 1)
        fin11()
        phase2_compute(0)
        phase2_prefetch(3, 0)
        phase2_prefetch(3, 1)
        phase2_compute(1)
        phase2_prefetch(4, 0)
        phase2_prefetch(4, 1)
        phase2_compute(2)
        phase2_prefetch(5, 0)
        phase2_prefetch(5, 1)
        phase2_compute(3)
        phase2_prefetch(6, 0)
        phase2_prefetch(6, 1)
        phase2_compute(4)
        phase2_prefetch(7, 0)
        phase2_prefetch(7, 1)
        phase2_compute(5)
        phase2_compute(6)
        phase2_compute(7)

    nc.compile()
    return nc


def kernel(hidden_states, attention_mask, wq, wk, wv, wo):
    global LAST_EXEC_NS
    bf16 = ml_dtypes.bfloat16

    hidden_states = np.asarray(hidden_states, dtype=np.float32)
    wq = np.asarray(wq, dtype=np.float32)
    wk = np.asarray(wk, dtype=np.float32)
    wv = np.asarray(wv, dtype=np.float32)
    wo = np.asarray(wo, dtype=np.float32)

    x = hidden_states.reshape(T, HID)
    # pretiled so every DMA reads contiguous per-partition chunks:
    # xT[p, tb, kt, c] = x[tb*512 + c, kt*128 + p]
    xTt = np.ascontiguousarray(
        x.reshape(N_TB, TOK_BLK, HID // 128, 128).transpose(3, 0, 2, 1)
    ).astype(bf16)
    cosT, sinT = _rope_tables()
    cosT16, sinT16 = cosT.astype(bf16), sinT.astype(bf16)
    k_idx = np.arange(KB)[:, None]
    q_idx = np.arange(KB)[None, :]
    binmask16 = (k_idx <= q_idx).astype(np.float16)

    def tile_w(w):   # [DL, HID] -> wT tiled [128, KT, DL]
        return np.ascontiguousarray(
            w.T.reshape(HID // 128, 128, DL).transpose(1, 0, 2)).astype(bf16)

    scale = np.float32(1.0 / np.sqrt(HD))
    in_maps = []
    for c in range(NC):
        rows = slice(c * DL, (c + 1) * DL)
        in_maps.append({
            "xT": xTt,
            "wqT": tile_w(wq[rows, :] * scale),
            "wkT": tile_w(wk[rows, :]),
            "wvT": tile_w(wv[rows, :]),
            "woT": tile_w(wo[rows, :]),
            "cosT": cosT16,
            "sinT": sinT16,
            "masks": binmask16,
        })

    if "nc" not in _CACHE:
        _CACHE["nc"] = _build()
    nc = _CACHE["nc"]

    res = run_bass_kernel_spmd(nc, in_maps, core_ids=list(range(NC)))
    LAST_EXEC_NS = res.exec_time_ns

    outT = np.concatenate([np.asarray(res.results[c]["out"]) for c in range(NC)],
                          axis=0)                          # [HID, T]
    return np.ascontiguousarray(outT.T).reshape(B, S, HID).astype(np.float32)
